# revision 1
# baseline (speedup 1.0000x reference)
"""Trainium2 Bass kernel for nn_InpaintContextAttentionUnit.

Per-sample computation (B=8 samples -> 1 per NeuronCore):
  fm [512,512,16] -> avgpool(64x2) -> pooled [8,256,16]
  -> two masked 3x3 convs (middle row / middle col of kernel zeroed) + bias + relu
  -> bilinear upsample back to [512,512,16] (separable; half-pixel centers, edge clamp)
  -> out [512,512,48] = concat(fm, fm - row_up, fm - col_up)

Design:
  - pooling: PE matmul with a [128,2] block-mean matrix (H-reduce); W-pair add
    folded into a 2-matmul PSUM accumulation (even/odd x, strided rhs)
  - conv: per (branch, n-pair chunk): zero-init matmul + ~6 accumulating
    [16c,16f]x[16c,<=512] matmuls in PSUM; relu+bias on ACT; taps read from a
    wp-halo'd [16c, 8n x 258wp] buffer assembled via a DRAM bounce
  - W-upsample (x2, weights .25/.75): 2 strided scalar_tensor_tensor ops over an
    edge-replicated halo buffer
  - H-upsample (x64): PE matmul rw[8n, x] with host-built HUp interp matrix
    (row branch at partitions 0-7, col branch at 32-39 per base-partition rules)
  - combine: DVE subtract (fm - psum, strided APs) + ACT copy into interleaved
    [y, x, 48ch] staging tiles, contiguous 3 MiB DMAs out
  - the pooled->conv->upsample chain runs in bf16 (PE bf16 is ~4x faster than the
    fp32-emulation path); PSUM accumulation, fm passthrough, subtract, and the
    output stay fp32
All constant matrices are precomputed on host and passed as extra inputs.
"""

import numpy as np
import ml_dtypes

H, W, C, F = 512, 512, 16, 16
NPOOL = 8
WP = W // 2  # 256
CH_OUT = 3 * C  # 48

_cache = {}


def _host_consts(kernel, bias):
    """Build host-side constant matrices (bf16 for the PE-side constants)."""
    bf = ml_dtypes.bfloat16
    # pooling weights: [128, 2], 1/128 (exact in bf16) where row block matches
    poolw = np.zeros((128, 2), np.float32)
    poolw[:64, 0] = 1.0 / 128.0
    poolw[64:, 1] = 1.0 / 128.0
    # H-upsample matrix: hup[n, y] = weight of pooled row n for output row y
    # (k/64 weights are exact in bf16)
    hup = np.zeros((NPOOL, H), np.float32)
    scale = H // NPOOL
    for y in range(H):
        yf = (y + 0.5) / scale - 0.5
        i0 = int(np.floor(yf))
        w = yf - i0
        hup[min(max(i0, 0), NPOOL - 1), y] += 1.0 - w
        hup[min(max(i0 + 1, 0), NPOOL - 1), y] += w
    hup2 = np.zeros((40, H), np.float32)
    hup2[0:8] = hup
    hup2[32:40] = hup  # col-branch copy at base partition 32
    # conv taps: branch 0 (row conv): K[dn+1, dwp+1]; branch 1 (col): K[dwp+1, dn+1]
    taps0 = [(dn, dwp) for dn in (-1, 1) for dwp in (-1, 0, 1)]
    taps1 = [(dn, dwp) for dwp in (-1, 1) for dn in (-1, 0, 1)]
    kt = np.zeros((16, 13 * 16), np.float32)  # [c, tap*16+f]; slot 12 = zeros
    for i, (dn, dwp) in enumerate(taps0):
        kt[:, i * 16:(i + 1) * 16] = kernel[dn + 1, dwp + 1]
    for i, (dn, dwp) in enumerate(taps1):
        kt[:, (6 + i) * 16:(7 + i) * 16] = kernel[dwp + 1, dn + 1]
    bias2 = np.ascontiguousarray(bias.reshape(16, 1)).astype(np.float32)
    return (poolw.astype(bf), hup2.astype(bf), kt.astype(bf), bias2, taps0, taps1)


def _build_program(compile=True):
    import concourse.bass as bass
    import concourse.bacc as bacc
    import concourse.mybir as mybir
    import concourse.tile as tile

    dt = mybir.dt.float32
    db = mybir.dt.bfloat16
    nc = bacc.Bacc()

    fm_d = nc.declare_dram_parameter("feature_map", [H, W, C], dt, isOutput=False)
    poolw_d = nc.declare_dram_parameter("poolw", [128, 2], db, isOutput=False)
    hup_d = nc.declare_dram_parameter("hup", [40, H], db, isOutput=False)
    ktaps_d = nc.declare_dram_parameter("ktaps", [16, 208], db, isOutput=False)
    bias_d = nc.declare_dram_parameter("bias2", [16, 1], dt, isOutput=False)
    out_d = nc.declare_dram_parameter("out", [H, W, CH_OUT], dt, isOutput=True)

    taps0 = [(dn, dwp) for dn in (-1, 1) for dwp in (-1, 0, 1)]
    taps1 = [(dn, dwp) for dwp in (-1, 1) for dn in (-1, 0, 1)]
    taps_by_branch = [taps0, taps1]

    with tile.TileContext(nc) as tc:
        with (
            tc.tile_pool(name="consts", bufs=1) as cpool,
            tc.tile_pool(name="fm", bufs=2) as fmpool,
            tc.tile_pool(name="persist", bufs=1) as ppool,
        ):
            # ---- load constants ----
            poolw_t = cpool.tile([128, 2], db)
            nc.sync.dma_start(out=poolw_t[:], in_=poolw_d[:])
            hup_t = cpool.tile([40, H], db)
            nc.sync.dma_start(out=hup_t[:], in_=hup_d[:])
            ktaps_t = cpool.tile([16, 208], db)
            nc.sync.dma_start(out=ktaps_t[:], in_=ktaps_d[:])
            bias_t = cpool.tile([16, 1], dt)
            nc.sync.dma_start(out=bias_t[:], in_=bias_d[:])

            # rw [40, (16 f, 512 x)] bf16: partitions 0-7 row-branch, 32-39 col-branch
            rw_t = ppool.tile([40, 16 * 512], db)

            # ================= PASS A: pooling + conv + W-upsample =================
            with (
                tc.tile_pool(name="passA", bufs=1) as apool,
                tc.tile_pool(name="dram", bufs=1, space="DRAM") as dpool,
            ):
                # pooled_T [16 c, (8 n, 258 wp)] bf16, zero wp-halo; n-direction
                # zero-padding handled by clipped matmul n-ranges
                tpad_t = apool.tile([16, NPOOL * 258], db)
                tpad3 = tpad_t[:].rearrange("p (n w) -> p n w", w=258)

                # pooled_ncw [8 n, (16 c, 256 wp)] bf16, c-major
                ncw_t = apool.tile([NPOOL, 16 * WP], db)

                with tc.tile_pool(name="psA", bufs=1, space="PSUM") as psA:
                    for t in range(4):
                        # bf16 copy of fm for pooling only (SWDGE cast-DMA)
                        fmb_t = apool.tile([128, W * C], db, tag="fmA", bufs=4)
                        fmb3 = fmb_t[:].rearrange("p (x c) -> p x c", c=C)
                        nc.gpsimd.dma_start(out=fmb3, in_=fm_d[128 * t:128 * (t + 1)])

                        # stage [2, (c, wp)] bf16 on partitions 0-1
                        stage_t = apool.tile([2, 16 * WP], db, tag="stage", bufs=2)
                        stage3 = stage_t[:].rearrange("p (c w) -> p w c", c=16)
                        # fm viewed (xp, parity, c): W-pair add in PE accumulation;
                        # one whole-PSUM [2, 4096] tile per fm tile, each j-block
                        # lands in its own bank (512 f32 = 1 bank)
                        fmr = fmb_t[:].rearrange("p (xp two c) -> p xp two c", two=2, c=16)
                        ps = psA.tile([2, 8 * 512], dt, tag="pool")
                        for j in range(8):  # 32-xp chunks -> N=512
                            for par in range(2):
                                nc.tensor.matmul(
                                    ps[:, 512 * j:512 * (j + 1)], poolw_t[:],
                                    fmr[:, 32 * j:32 * (j + 1), par, :],
                                    start=(par == 0), stop=(par == 1),
                                )
                        ps3 = ps[:].rearrange("p (xp c) -> p xp c", c=16)
                        nc.vector.tensor_copy(stage3, ps3)
                        nc.sync.dma_start(out=ncw_t[2 * t:2 * t + 2, :], in_=stage_t[:])

                # pooled_ncw -> pooled_T (c to partitions) via DRAM bounce, adding
                # zero wp-halo columns (zeros sourced from hup rows 8-15, zero by
                # construction)
                ncw_dram = dpool.tile([NPOOL, 16 * 258], db)
                nd3 = ncw_dram[:].rearrange("n (c w) -> n c w", w=258)
                ncw3s = ncw_t[:].rearrange("p (c w) -> p c w", w=WP)
                nc.sync.dma_start(out=nd3[:, :, 1:257], in_=ncw3s)
                zsrc = hup_d[8:16, 0:16]  # [8, 16] zeros
                nc.sync.dma_start(out=nd3[:, :, 0:1], in_=zsrc)
                nc.sync.dma_start(out=nd3[:, :, 257:258], in_=zsrc)
                ncwd3 = ncw_dram[:].rearrange("n (c w) -> c n w", w=258)
                nc.sync.dma_start(out=tpad3, in_=ncwd3)

                # ---- conv branches ----
                conv_t = apool.tile([16, 2 * NPOOL * WP], db, tag="conv_t")
                psC_cm = tc.tile_pool(name="psConv", bufs=4, space="PSUM")
                psC_pool = psC_cm.__enter__()
                for b in range(2):
                    for ch in range(4):  # n-pair chunks: n in {2ch, 2ch+1}
                        n0 = 2 * ch
                        ps = psC_pool.tile([16, 2 * WP], dt, tag="conv")
                        # zero-init whole chunk (ktaps slot 12 = zeros)
                        nc.tensor.matmul(
                            ps[:], ktaps_t[:, 192:208], tpad3[:, n0:n0 + 2, 1:257],
                            start=True, stop=False, skip_group_check=True,
                        )
                        pieces = []
                        for i, (dn, dwp) in enumerate(taps_by_branch[b]):
                            nlo = max(n0, -dn)
                            nhi = min(n0 + 2, NPOOL - dn)
                            if nhi <= nlo:
                                continue
                            pieces.append((b * 6 + i, dn, dwp, nlo, nhi))
                        for k, (sl, dn, dwp, nlo, nhi) in enumerate(pieces):
                            nc.tensor.matmul(
                                ps[:, (nlo - n0) * WP:(nhi - n0) * WP],
                                ktaps_t[:, sl * 16:(sl + 1) * 16],
                                tpad3[:, nlo + dn:nhi + dn, 1 + dwp:257 + dwp],
                                start=False, stop=(k == len(pieces) - 1),
                                skip_group_check=True,
                            )
                        nc.scalar.activation(
                            out=conv_t[:, (b * NPOOL + n0) * WP:(b * NPOOL + n0 + 2) * WP],
                            in_=ps[:],
                            func=mybir.ActivationFunctionType.Relu,
                            bias=bias_t[:, 0:1],
                        )

                psC_cm.__exit__(None, None, None)
                # conv [16 f, (b, n, wp)] -> rop_pad [(b,n) parts, (16 f, 258 wp)]
                # via DRAM bounce (keeps consumer sync fan-in small)
                rop_t = apool.tile([40, 16 * 258], db)
                rop3 = rop_t[:].rearrange("p (f w) -> p f w", w=258)
                conv_dram = dpool.tile([16, 2 * NPOOL * WP], db)
                nc.sync.dma_start(out=conv_dram[:], in_=conv_t[:])
                cd4 = conv_dram[:].rearrange("f (b n w) -> b n f w", b=2, n=NPOOL)
                for b in range(2):
                    pg = 32 * b  # partition base: row->0, col->32
                    nc.sync.dma_start(out=rop3[pg:pg + 8, :, 1:257], in_=cd4[b])
                # edge replicate (W clamp)
                for pg in (0, 32):
                    nc.vector.tensor_copy(rop3[pg:pg + 8, :, 0:1], rop3[pg:pg + 8, :, 1:2])
                    nc.vector.tensor_copy(rop3[pg:pg + 8, :, 257:258], rop3[pg:pg + 8, :, 256:257])

                # W-upsample: rw[., f, 2k]   = 0.25*pad[k]   + 0.75*pad[k+1]
                #             rw[., f, 2k+1] = 0.25*pad[k+2] + 0.75*pad[k+1]
                t75_t = apool.tile([40, 16 * 258], db, tag="conv_t")
                t753 = t75_t[:].rearrange("p (f w) -> p f w", w=258)
                rw4 = rw_t[:].rearrange("p (f x two) -> p f x two", two=2, x=WP)
                for pg, eng in ((0, nc.vector), (32, nc.vector)):
                    eng.tensor_scalar_mul(
                        t75_t[pg:pg + 8, :], rop_t[pg:pg + 8, :], 0.75)
                    eng.scalar_tensor_tensor(
                        out=rw4[pg:pg + 8, :, :, 0],
                        in0=rop3[pg:pg + 8, :, 0:256],
                        scalar=0.25,
                        in1=t753[pg:pg + 8, :, 1:257],
                        op0=mybir.AluOpType.mult,
                        op1=mybir.AluOpType.add,
                    )
                    eng.scalar_tensor_tensor(
                        out=rw4[pg:pg + 8, :, :, 1],
                        in0=rop3[pg:pg + 8, :, 2:258],
                        scalar=0.25,
                        in1=t753[pg:pg + 8, :, 1:257],
                        op0=mybir.AluOpType.mult,
                        op1=mybir.AluOpType.add,
                    )

            # ================= PASS B: H-upsample + combine + store =================
            with (
                tc.tile_pool(name="passB", bufs=1) as bpool,
                tc.tile_pool(name="psB", bufs=2, space="PSUM") as psB,
            ):
                rwx = rw_t[:].rearrange("p (f x) -> p f x", x=W)
                for t in range(4):
                    fm_t = fmpool.tile([128, W * C], dt, tag="fm")
                    fm3 = fm_t[:].rearrange("p (x c) -> p x c", c=C)
                    nc.sync.dma_start(out=fm3, in_=fm_d[128 * t:128 * (t + 1)])

                    outqs = []
                    for q in range(4):
                        outq_t = bpool.tile([128, 128 * CH_OUT], dt, tag=f"out{q}")
                        outq3 = outq_t[:].rearrange("p (x ch) -> p x ch", ch=CH_OUT)
                        nc.scalar.activation(
                            out=outq3[:, :, 0:16],
                            in_=fm3[:, 128 * q:128 * (q + 1), :],
                            func=mybir.ActivationFunctionType.Copy,
                        )
                        outqs.append(outq3)

                    for b in range(2):
                        pg = 32 * b
                        lhsT = hup_t[pg:pg + 8, 128 * t:128 * (t + 1)]  # [8, 128]
                        for fq in range(4):  # f-quads
                            ps = psB.tile([128, 4 * W], dt, tag="up")
                            psf = ps[:].rearrange("p (f x) -> p f x", x=W)
                            for fi in range(4):
                                nc.tensor.matmul(
                                    psf[:, fi, :],
                                    lhsT,
                                    rwx[pg:pg + 8, fq * 4 + fi, :],
                                    start=True, stop=True,
                                )
                            psx = ps[:].rearrange("p (f x) -> p x f", x=W)
                            for q in range(4):
                                nc.vector.tensor_sub(
                                    outqs[q][:, :, 16 * (b + 1) + 4 * fq:
                                             16 * (b + 1) + 4 * fq + 4],
                                    fm3[:, 128 * q:128 * (q + 1), 4 * fq:4 * fq + 4],
                                    psx[:, 128 * q:128 * (q + 1), :],
                                )
                    for q in range(4):
                        nc.sync.dma_start(
                            out=out_d[128 * t:128 * (t + 1), 128 * q:128 * (q + 1), :],
                            in_=outqs[q],
                        )
    if compile:
        nc.compile()
    return nc


def _get_program():
    if "nc" not in _cache:
        _cache["nc"] = _build_program()
    return _cache["nc"]


def kernel(feature_map, kernel, bias):
    from concourse.bass_utils import run_bass_kernel_spmd

    feature_map = np.ascontiguousarray(feature_map, dtype=np.float32)
    kernel = np.ascontiguousarray(kernel, dtype=np.float32)
    bias = np.ascontiguousarray(bias, dtype=np.float32)
    B = feature_map.shape[0]
    assert B == 8

    poolw, hup, kt, bias2, _, _ = _host_consts(kernel, bias)
    nc = _get_program()
    in_maps = [
        {
            "feature_map": feature_map[b],
            "poolw": poolw,
            "hup": hup,
            "ktaps": kt,
            "bias2": bias2,
        }
        for b in range(B)
    ]
    res = run_bass_kernel_spmd(nc, in_maps, list(range(B)))
    out = np.stack([res.results[b]["out"] for b in range(B)])
    return out



# revision 13
# speedup vs baseline: 1.5839x; 1.5839x over previous
"""Trainium2 Bass kernel for nn_InpaintContextAttentionUnit.

Per-sample computation (B=8 samples -> 1 per NeuronCore):
  fm [512,512,16] -> avgpool(64x2) -> pooled [8,256,16]
  -> two masked 3x3 convs (middle row / middle col of kernel zeroed) + bias + relu
  -> bilinear upsample back to [512,512,16] (separable; half-pixel centers, edge clamp)
  -> out [512,512,48] = concat(fm, fm - row_up, fm - col_up)

Design (v2):
  - fm is read from HBM ONCE per core as a resident bf16 SBUF copy (SWDGE
    cast-DMA); pooling, the passthrough channels, and the subtract all read it
    (bf16 rounding of fm costs ~1e-2 abs vs the 0.109 tolerance)
  - pooling: PE matmul with a [128,2] block-mean matrix (H-reduce); W-pair add
    folded into a 2-matmul PSUM accumulation (even/odd x, strided rhs); all 4
    fm tiles accumulate into one [8, 4096] PSUM tile drained by one ACT copy
  - conv: per (branch, n-pair chunk): zero-init matmul + ~6 accumulating
    [16c,16f]x[16c,<=512] matmuls in PSUM; relu+bias on ACT; taps read from a
    wp-halo'd [16c, 8n x 258wp] buffer assembled via a DRAM bounce
  - W-upsample (x2, weights .25/.75): 2 strided scalar_tensor_tensor ops over an
    edge-replicated halo buffer
  - H-upsample (x64): PE matmul rw[8n, x] with host-built HUp interp matrix
    (row branch at partitions 0-7, col branch at 32-39 per base-partition rules)
  - combine: DVE subtract (fm - psum, strided APs) + ACT copy into interleaved
    [y, x, 48ch] staging tiles, contiguous 3 MiB DMAs out
  - the pooled->conv->upsample chain runs in bf16 (PE bf16 is ~4x faster than the
    fp32-emulation path); PSUM accumulation, fm passthrough, subtract, and the
    output stay fp32
All constant matrices are precomputed on host and passed as extra inputs.
"""

import numpy as np
import ml_dtypes

H, W, C, F = 512, 512, 16, 16
NPOOL = 8
WP = W // 2  # 256
CH_OUT = 3 * C  # 48

_cache = {}


def _host_consts(kernel, bias):
    """Build host-side constant matrices (bf16 for the PE-side constants)."""
    bf = ml_dtypes.bfloat16
    # pooling weights: [128, 32]; tile t uses columns 8t:8t+8, whose cols
    # 2t/2t+1 hold 1/128 (exact in bf16) on the matching 64-row block and all
    # other cols are zero -> a [128,8]-lhsT matmul per tile writes the full
    # 8-partition PSUM tile (base-partition rule) while accumulating only
    # its own pooled rows
    poolw = np.zeros((128, 32), np.float32)
    for t in range(4):
        poolw[:64, 8 * t + 2 * t] = 1.0 / 128.0
        poolw[64:, 8 * t + 2 * t + 1] = 1.0 / 128.0
    # H-upsample matrix: hup[n, y] = weight of pooled row n for output row y
    # (k/64 weights are exact in bf16)
    hup = np.zeros((NPOOL, H), np.float32)
    scale = H // NPOOL
    for y in range(H):
        yf = (y + 0.5) / scale - 0.5
        i0 = int(np.floor(yf))
        w = yf - i0
        hup[min(max(i0, 0), NPOOL - 1), y] += 1.0 - w
        hup[min(max(i0 + 1, 0), NPOOL - 1), y] += w
    hup2 = np.zeros((40, H), np.float32)
    hup2[0:8] = hup
    hup2[32:40] = hup  # col-branch copy at base partition 32
    # conv taps: branch 0 (row conv): K[dn+1, dwp+1]; branch 1 (col): K[dwp+1, dn+1]
    taps0 = [(dn, dwp) for dn in (-1, 1) for dwp in (-1, 0, 1)]
    taps1 = [(dn, dwp) for dwp in (-1, 1) for dn in (-1, 0, 1)]
    kt = np.zeros((16, 13 * 16), np.float32)  # [c, tap*16+f]; slot 12 = zeros
    for i, (dn, dwp) in enumerate(taps0):
        kt[:, i * 16:(i + 1) * 16] = kernel[dn + 1, dwp + 1]
    for i, (dn, dwp) in enumerate(taps1):
        kt[:, (6 + i) * 16:(7 + i) * 16] = kernel[dwp + 1, dn + 1]
    bias2 = np.ascontiguousarray(bias.reshape(16, 1)).astype(np.float32)
    return (poolw.astype(bf), hup2.astype(bf), kt.astype(bf), bias2, taps0, taps1)


def _build_program(compile=True):
    import concourse.bass as bass
    import concourse.bacc as bacc
    import concourse.mybir as mybir
    import concourse.tile as tile

    dt = mybir.dt.float32
    db = mybir.dt.bfloat16
    nc = bacc.Bacc()

    fm_d = nc.declare_dram_parameter("feature_map", [H, W, C], dt, isOutput=False)
    poolw_d = nc.declare_dram_parameter("poolw", [128, 32], db, isOutput=False)
    hup_d = nc.declare_dram_parameter("hup", [40, H], db, isOutput=False)
    ktaps_d = nc.declare_dram_parameter("ktaps", [16, 208], db, isOutput=False)
    bias_d = nc.declare_dram_parameter("bias2", [16, 1], dt, isOutput=False)
    out_d = nc.declare_dram_parameter("out", [H, W, CH_OUT], dt, isOutput=True)

    taps0 = [(dn, dwp) for dn in (-1, 1) for dwp in (-1, 0, 1)]
    taps1 = [(dn, dwp) for dwp in (-1, 1) for dn in (-1, 0, 1)]
    taps_by_branch = [taps0, taps1]

    with tile.TileContext(nc) as tc:
        with (
            tc.tile_pool(name="consts", bufs=1) as cpool,
            tc.tile_pool(name="persist", bufs=1) as ppool,
        ):
            # ---- load constants ----
            poolw_t = cpool.tile([128, 32], db)
            nc.sync.dma_start(out=poolw_t[:], in_=poolw_d[:])
            hup_t = cpool.tile([40, H], db)
            nc.sync.dma_start(out=hup_t[:], in_=hup_d[:])
            ktaps_t = cpool.tile([16, 208], db)
            nc.sync.dma_start(out=ktaps_t[:], in_=ktaps_d[:])
            bias_t = cpool.tile([16, 1], dt)
            nc.sync.dma_start(out=bias_t[:], in_=bias_d[:])

            # rw [40, (16 f, 512 x)] bf16: partitions 0-7 row-branch, 32-39 col-branch
            rw_t = ppool.tile([40, 16 * 512], db)

            # resident bf16 copy of fm (single HBM read serves pooling + pass B)
            fmb_ts = []
            for t in range(4):
                fmb_t = ppool.tile([128, W * C], db, tag=f"fmb{t}")
                fmb3 = fmb_t[:].rearrange("p (x c) -> p x c", c=C)
                nc.gpsimd.dma_start(out=fmb3, in_=fm_d[128 * t:128 * (t + 1)])
                fmb_ts.append(fmb_t)

            # ================= PASS A: pooling + conv + W-upsample =================
            with (
                tc.tile_pool(name="passA", bufs=1) as apool,
                tc.tile_pool(name="dram", bufs=1, space="DRAM") as dpool,
            ):
                # pooled_T [16 c, (8 n, 258 wp)] bf16, zero wp-halo; n-direction
                # zero-padding handled by clipped matmul n-ranges
                tpad_t = apool.tile([16, NPOOL * 258], db)
                tpad3 = tpad_t[:].rearrange("p (n w) -> p n w", w=258)

                with tc.tile_pool(name="psA", bufs=1, space="PSUM") as psA:
                    # all 8 pooled rows accumulate into one [8, 4096] PSUM tile
                    # (tile t's matmuls land on partitions 2t:2t+2), so one wide
                    # copy drains it instead of four 2-partition copies
                    ps8 = psA.tile([8, 8 * 512], dt, tag="pool")
                    for t in range(4):
                        fmr = fmb_ts[t][:].rearrange(
                            "p (xp two c) -> p xp two c", two=2, c=16)
                        for j in range(8):  # 32-xp chunks -> N=512
                            for par in range(2):
                                nc.tensor.matmul(
                                    ps8[:, 512 * j:512 * (j + 1)],
                                    poolw_t[:, 8 * t:8 * (t + 1)],
                                    fmr[:, 32 * j:32 * (j + 1), par, :],
                                    start=(t == 0 and par == 0),
                                    stop=(t == 3 and par == 1),
                                    skip_group_check=True,
                                )
                    # stage [8 n, (16 c, 256 wp)] bf16 (ACT copy+cast, 8 lanes)
                    stage_t = apool.tile([NPOOL, 16 * WP], db)
                    stage3 = stage_t[:].rearrange("p (c w) -> p w c", c=16)
                    ps83 = ps8[:].rearrange("p (xp c) -> p xp c", c=16)
                    nc.scalar.activation(
                        out=stage3, in_=ps83,
                        func=mybir.ActivationFunctionType.Copy)

                # pooled -> pooled_T (c to partitions) via DRAM bounce, adding
                # zero wp-halo columns (zeros sourced from hup rows 8-15, zero by
                # construction)
                ncw_dram = dpool.tile([NPOOL, 16 * 258], db)
                nd3 = ncw_dram[:].rearrange("n (c w) -> n c w", w=258)
                ncw3s = stage_t[:].rearrange("p (c w) -> p c w", w=WP)
                nc.sync.dma_start(out=nd3[:, :, 1:257], in_=ncw3s)
                zsrc = hup_d[8:16, 0:16]  # [8, 16] zeros
                nc.sync.dma_start(out=nd3[:, :, 0:1], in_=zsrc)
                nc.sync.dma_start(out=nd3[:, :, 257:258], in_=zsrc)
                ncwd3 = ncw_dram[:].rearrange("n (c w) -> c n w", w=258)
                nc.sync.dma_start(out=tpad3, in_=ncwd3)

                # ---- conv branches ----
                conv_t = apool.tile([16, 2 * NPOOL * WP], db, tag="conv_t")
                psC_cm = tc.tile_pool(name="psConv", bufs=4, space="PSUM")
                psC_pool = psC_cm.__enter__()
                for b in range(2):
                    for ch in range(4):  # n-pair chunks: n in {2ch, 2ch+1}
                        n0 = 2 * ch
                        ps = psC_pool.tile([16, 2 * WP], dt, tag="conv")
                        # zero-init whole chunk (ktaps slot 12 = zeros)
                        nc.tensor.matmul(
                            ps[:], ktaps_t[:, 192:208], tpad3[:, n0:n0 + 2, 1:257],
                            start=True, stop=False, skip_group_check=True,
                        )
                        pieces = []
                        for i, (dn, dwp) in enumerate(taps_by_branch[b]):
                            nlo = max(n0, -dn)
                            nhi = min(n0 + 2, NPOOL - dn)
                            if nhi <= nlo:
                                continue
                            pieces.append((b * 6 + i, dn, dwp, nlo, nhi))
                        for k, (sl, dn, dwp, nlo, nhi) in enumerate(pieces):
                            nc.tensor.matmul(
                                ps[:, (nlo - n0) * WP:(nhi - n0) * WP],
                                ktaps_t[:, sl * 16:(sl + 1) * 16],
                                tpad3[:, nlo + dn:nhi + dn, 1 + dwp:257 + dwp],
                                start=False, stop=(k == len(pieces) - 1),
                                skip_group_check=True,
                            )
                        nc.scalar.activation(
                            out=conv_t[:, (b * NPOOL + n0) * WP:(b * NPOOL + n0 + 2) * WP],
                            in_=ps[:],
                            func=mybir.ActivationFunctionType.Relu,
                            bias=bias_t[:, 0:1],
                        )

                psC_cm.__exit__(None, None, None)
                # conv [16 f, (b, n, wp)] -> rop_pad [(b,n) parts, (16 f, 258 wp)]
                # via DRAM bounce (keeps consumer sync fan-in small)
                rop_t = apool.tile([40, 16 * 258], db)
                rop3 = rop_t[:].rearrange("p (f w) -> p f w", w=258)
                conv_dram = dpool.tile([16, 2 * NPOOL * WP], db)
                nc.sync.dma_start(out=conv_dram[:], in_=conv_t[:])
                cd4 = conv_dram[:].rearrange("f (b n w) -> b n f w", b=2, n=NPOOL)
                for b in range(2):
                    pg = 32 * b  # partition base: row->0, col->32
                    nc.sync.dma_start(out=rop3[pg:pg + 8, :, 1:257], in_=cd4[b])
                # edge replicate (W clamp)
                for pg in (0, 32):
                    nc.vector.tensor_copy(rop3[pg:pg + 8, :, 0:1], rop3[pg:pg + 8, :, 1:2])
                    nc.vector.tensor_copy(rop3[pg:pg + 8, :, 257:258], rop3[pg:pg + 8, :, 256:257])

                # W-upsample: rw[., f, 2k]   = 0.25*pad[k]   + 0.75*pad[k+1]
                #             rw[., f, 2k+1] = 0.25*pad[k+2] + 0.75*pad[k+1]
                t75_t = apool.tile([40, 16 * 258], db, tag="conv_t")
                t753 = t75_t[:].rearrange("p (f w) -> p f w", w=258)
                rw4 = rw_t[:].rearrange("p (f x two) -> p f x two", two=2, x=WP)
                for pg, eng in ((0, nc.vector), (32, nc.vector)):
                    eng.tensor_scalar_mul(
                        t75_t[pg:pg + 8, :], rop_t[pg:pg + 8, :], 0.75)
                    eng.scalar_tensor_tensor(
                        out=rw4[pg:pg + 8, :, :, 0],
                        in0=rop3[pg:pg + 8, :, 0:256],
                        scalar=0.25,
                        in1=t753[pg:pg + 8, :, 1:257],
                        op0=mybir.AluOpType.mult,
                        op1=mybir.AluOpType.add,
                    )
                    eng.scalar_tensor_tensor(
                        out=rw4[pg:pg + 8, :, :, 1],
                        in0=rop3[pg:pg + 8, :, 2:258],
                        scalar=0.25,
                        in1=t753[pg:pg + 8, :, 1:257],
                        op0=mybir.AluOpType.mult,
                        op1=mybir.AluOpType.add,
                    )

            # ================= PASS B: H-upsample + combine + store =================
            # q-outer: one [128, 128x, 48ch] staging tile at a time (bufs=2 for
            # overlap), per-(q,b,fq) single-bank PSUM tiles (free dim 128)
            with (
                tc.tile_pool(name="passB", bufs=2) as bpool,
                tc.tile_pool(name="psB", bufs=2, space="PSUM") as psB,
            ):
                rwx = rw_t[:].rearrange("p (f x) -> p f x", x=W)
                for t in range(4):
                    fm3 = fmb_ts[t][:].rearrange("p (x c) -> p x c", c=C)
                    for q in range(4):
                        xs = 128 * q
                        outq_t = bpool.tile([128, 128 * CH_OUT], dt, tag="outq")
                        outq3 = outq_t[:].rearrange("p (x ch) -> p x ch", ch=CH_OUT)
                        nc.scalar.activation(
                            out=outq3[:, :, 0:16],
                            in_=fm3[:, xs:xs + 128, :],
                            func=mybir.ActivationFunctionType.Copy,
                        )
                        for b in range(2):
                            pg = 32 * b
                            lhsT = hup_t[pg:pg + 8, 128 * t:128 * (t + 1)]  # [8,128]
                            # one 4-bank PSUM tile [128, (16 f, 128 x)]; one
                            # matmul per bank (rhs free = 4f x 128x strided)
                            ps = psB.tile([128, 16 * 128], dt, tag="up")
                            psf = ps[:].rearrange("p (f x) -> p f x", x=128)
                            for fq in range(4):
                                nc.tensor.matmul(
                                    psf[:, 4 * fq:4 * (fq + 1), :],
                                    lhsT,
                                    rwx[pg:pg + 8, fq * 4:fq * 4 + 4, xs:xs + 128],
                                    start=True, stop=True,
                                )
                            psx = ps[:].rearrange("p (f x) -> p x f", x=128)
                            nc.vector.tensor_sub(
                                outq3[:, :, 16 * (b + 1):16 * (b + 2)],
                                fm3[:, xs:xs + 128, :],
                                psx[:],
                            )
                        nc.sync.dma_start(
                            out=out_d[128 * t:128 * (t + 1), xs:xs + 128, :],
                            in_=outq3,
                        )
    if compile:
        nc.compile()
    return nc


def _get_program():
    if "nc" not in _cache:
        _cache["nc"] = _build_program()
    return _cache["nc"]


def kernel(feature_map, kernel, bias):
    from concourse.bass_utils import run_bass_kernel_spmd

    feature_map = np.ascontiguousarray(feature_map, dtype=np.float32)
    kernel = np.ascontiguousarray(kernel, dtype=np.float32)
    bias = np.ascontiguousarray(bias, dtype=np.float32)
    B = feature_map.shape[0]
    assert B == 8

    poolw, hup, kt, bias2, _, _ = _host_consts(kernel, bias)
    nc = _get_program()
    in_maps = [
        {
            "feature_map": feature_map[b],
            "poolw": poolw,
            "hup": hup,
            "ktaps": kt,
            "bias2": bias2,
        }
        for b in range(B)
    ]
    res = run_bass_kernel_spmd(nc, in_maps, list(range(B)))
    out = np.stack([res.results[b]["out"] for b in range(B)])
    return out



# revision 17
# speedup vs baseline: 1.5918x; 1.0050x over previous
"""Trainium2 Bass kernel for nn_InpaintContextAttentionUnit.

Per-sample computation (B=8 samples -> 1 per NeuronCore):
  fm [512,512,16] -> avgpool(64x2) -> pooled [8,256,16]
  -> two masked 3x3 convs (middle row / middle col of kernel zeroed) + bias + relu
  -> bilinear upsample back to [512,512,16] (separable; half-pixel centers, edge clamp)
  -> out [512,512,48] = concat(fm, fm - row_up, fm - col_up)

Design (v2):
  - fm is read from HBM ONCE per core as a resident bf16 SBUF copy (SWDGE
    cast-DMA); pooling, the passthrough channels, and the subtract all read it
    (bf16 rounding of fm costs ~1e-2 abs vs the 0.109 tolerance)
  - pooling: PE matmul with a [128,2] block-mean matrix (H-reduce); W-pair add
    folded into a 2-matmul PSUM accumulation (even/odd x, strided rhs); all 4
    fm tiles accumulate into one [8, 4096] PSUM tile drained by one ACT copy
  - conv: per (branch, n-pair chunk): zero-init matmul + ~6 accumulating
    [16c,16f]x[16c,<=512] matmuls in PSUM; relu+bias on ACT; taps read from a
    wp-halo'd [16c, 8n x 258wp] buffer assembled via a DRAM bounce
  - W-upsample (x2, weights .25/.75): 2 strided scalar_tensor_tensor ops over an
    edge-replicated halo buffer
  - H-upsample (x64): PE matmul rw[8n, x] with host-built HUp interp matrix
    (row branch at partitions 0-7, col branch at 32-39 per base-partition rules)
  - combine: DVE subtract (fm - psum, strided APs) + ACT copy into interleaved
    [y, x, 48ch] staging tiles, contiguous 3 MiB DMAs out
  - the pooled->conv->upsample chain runs in bf16 (PE bf16 is ~4x faster than the
    fp32-emulation path); PSUM accumulation, fm passthrough, subtract, and the
    output stay fp32
All constant matrices are precomputed on host and passed as extra inputs.
"""

import numpy as np
import ml_dtypes

H, W, C, F = 512, 512, 16, 16
NPOOL = 8
WP = W // 2  # 256
CH_OUT = 3 * C  # 48

_cache = {}


def _host_consts(kernel, bias):
    """Build host-side constant matrices (bf16 for the PE-side constants)."""
    bf = ml_dtypes.bfloat16
    # pooling weights: [128, 32]; tile t uses columns 8t:8t+8, whose cols
    # 2t/2t+1 hold 1/128 (exact in bf16) on the matching 64-row block and all
    # other cols are zero -> a [128,8]-lhsT matmul per tile writes the full
    # 8-partition PSUM tile (base-partition rule) while accumulating only
    # its own pooled rows
    poolw = np.zeros((128, 32), np.float32)
    for t in range(4):
        poolw[:64, 8 * t + 2 * t] = 1.0 / 128.0
        poolw[64:, 8 * t + 2 * t + 1] = 1.0 / 128.0
    # H-upsample matrix: hup[n, y] = weight of pooled row n for output row y
    # (k/64 weights are exact in bf16)
    hup = np.zeros((NPOOL, H), np.float32)
    scale = H // NPOOL
    for y in range(H):
        yf = (y + 0.5) / scale - 0.5
        i0 = int(np.floor(yf))
        w = yf - i0
        hup[min(max(i0, 0), NPOOL - 1), y] += 1.0 - w
        hup[min(max(i0 + 1, 0), NPOOL - 1), y] += w
    hup2 = np.zeros((40, H), np.float32)
    hup2[0:8] = hup
    hup2[32:40] = hup  # col-branch copy at base partition 32
    # conv taps: branch 0 (row conv): K[dn+1, dwp+1]; branch 1 (col): K[dwp+1, dn+1]
    taps0 = [(dn, dwp) for dn in (-1, 1) for dwp in (-1, 0, 1)]
    taps1 = [(dn, dwp) for dwp in (-1, 1) for dn in (-1, 0, 1)]
    kt = np.zeros((16, 13 * 16), np.float32)  # [c, tap*16+f]; slot 12 = zeros
    for i, (dn, dwp) in enumerate(taps0):
        kt[:, i * 16:(i + 1) * 16] = kernel[dn + 1, dwp + 1]
    for i, (dn, dwp) in enumerate(taps1):
        kt[:, (6 + i) * 16:(7 + i) * 16] = kernel[dwp + 1, dn + 1]
    bias2 = np.ascontiguousarray(bias.reshape(16, 1)).astype(np.float32)
    return (poolw.astype(bf), hup2.astype(bf), kt.astype(bf), bias2, taps0, taps1)


def _build_program(compile=True):
    import concourse.bass as bass
    import concourse.bacc as bacc
    import concourse.mybir as mybir
    import concourse.tile as tile

    dt = mybir.dt.float32
    db = mybir.dt.bfloat16
    nc = bacc.Bacc()

    fm_d = nc.declare_dram_parameter("feature_map", [H, W, C], dt, isOutput=False)
    poolw_d = nc.declare_dram_parameter("poolw", [128, 32], db, isOutput=False)
    hup_d = nc.declare_dram_parameter("hup", [40, H], db, isOutput=False)
    ktaps_d = nc.declare_dram_parameter("ktaps", [16, 208], db, isOutput=False)
    bias_d = nc.declare_dram_parameter("bias2", [16, 1], dt, isOutput=False)
    out_d = nc.declare_dram_parameter("out", [H, W, CH_OUT], dt, isOutput=True)

    taps0 = [(dn, dwp) for dn in (-1, 1) for dwp in (-1, 0, 1)]
    taps1 = [(dn, dwp) for dwp in (-1, 1) for dn in (-1, 0, 1)]
    taps_by_branch = [taps0, taps1]

    with tile.TileContext(nc) as tc:
        with (
            tc.tile_pool(name="consts", bufs=1) as cpool,
            tc.tile_pool(name="persist", bufs=1) as ppool,
        ):
            # ---- load constants ----
            poolw_t = cpool.tile([128, 32], db)
            nc.sync.dma_start(out=poolw_t[:], in_=poolw_d[:])
            hup_t = cpool.tile([40, H], db)
            nc.sync.dma_start(out=hup_t[:], in_=hup_d[:])
            ktaps_t = cpool.tile([16, 208], db)
            nc.sync.dma_start(out=ktaps_t[:], in_=ktaps_d[:])
            bias_t = cpool.tile([16, 1], dt)
            nc.sync.dma_start(out=bias_t[:], in_=bias_d[:])

            # rw [40, (16 f, 512 x)] bf16: partitions 0-7 row-branch, 32-39 col-branch
            rw_t = ppool.tile([40, 16 * 512], db)

            # resident bf16 copy of fm (single HBM read serves pooling + pass B)
            fmb_ts = []
            for t in range(4):
                fmb_t = ppool.tile([128, W * C], db, tag=f"fmb{t}")
                fmb3 = fmb_t[:].rearrange("p (x c) -> p x c", c=C)
                nc.gpsimd.dma_start(out=fmb3, in_=fm_d[128 * t:128 * (t + 1)])
                fmb_ts.append(fmb_t)

            # ================= PASS A: pooling + conv + W-upsample =================
            with (
                tc.tile_pool(name="passA", bufs=1) as apool,
                tc.tile_pool(name="dram", bufs=1, space="DRAM") as dpool,
            ):
                # pooled_T [16 c, (8 n, 258 wp)] bf16, zero wp-halo; n-direction
                # zero-padding handled by clipped matmul n-ranges
                tpad_t = apool.tile([16, NPOOL * 258], db)
                tpad3 = tpad_t[:].rearrange("p (n w) -> p n w", w=258)

                with tc.tile_pool(name="psA", bufs=1, space="PSUM") as psA:
                    # all 8 pooled rows accumulate into one [8, 4096] PSUM tile;
                    # rhs free AP is (c, xp) so PSUM lands (j, c, xp)-major and
                    # the drain copy below runs with 32-elem contiguous runs
                    ps8 = psA.tile([8, 8 * 512], dt, tag="pool")
                    for t in range(4):
                        fmc = fmb_ts[t][:].rearrange(
                            "p (xp two c) -> p c xp two", two=2, c=16)
                        for j in range(8):  # 32-xp chunks -> N=512
                            for par in range(2):
                                nc.tensor.matmul(
                                    ps8[:, 512 * j:512 * (j + 1)],
                                    poolw_t[:, 8 * t:8 * (t + 1)],
                                    fmc[:, :, 32 * j:32 * (j + 1), par],
                                    start=(t == 0 and par == 0),
                                    stop=(t == 3 and par == 1),
                                    skip_group_check=True,
                                )
                    # stage [8 n, (16 c, 256 w)] bf16, w = (j, xp); both APs
                    # have 32-elem contiguous inner runs
                    stage_t = apool.tile([NPOOL, 16 * WP], db)
                    stage4 = stage_t[:].rearrange(
                        "p (c j x) -> p c j x", c=16, j=8, x=32)
                    ps84 = ps8[:].rearrange(
                        "p (j c x) -> p c j x", j=8, c=16, x=32)
                    for j in range(8):  # per-j keeps canonical APs <= 3 dims
                        nc.vector.tensor_copy(stage4[:, :, j, :], ps84[:, :, j, :])

                # pooled -> pooled_T (c to partitions) via DRAM bounce, adding
                # zero wp-halo columns (zeros sourced from hup rows 8-15, zero by
                # construction)
                ncw_dram = dpool.tile([NPOOL, 16 * 258], db)
                nd3 = ncw_dram[:].rearrange("n (c w) -> n c w", w=258)
                ncw3s = stage_t[:].rearrange("p (c w) -> p c w", w=WP)
                nc.sync.dma_start(out=nd3[:, :, 1:257], in_=ncw3s)
                zsrc = hup_d[8:16, 0:16]  # [8, 16] zeros
                nc.sync.dma_start(out=nd3[:, :, 0:1], in_=zsrc)
                nc.sync.dma_start(out=nd3[:, :, 257:258], in_=zsrc)
                ncwd3 = ncw_dram[:].rearrange("n (c w) -> c n w", w=258)
                nc.sync.dma_start(out=tpad3, in_=ncwd3)

                # ---- conv branches (chunk-major so rw rows stream out early) ----
                # conv output kept on 16 f-partitions with a wp-halo:
                # c3 [16 f, (b, n, 258 wp)]; W-upsample runs on the same 16-lane
                # layout BEFORE the n-to-partition transpose, then each chunk's
                # rows bounce via DRAM into rw [n @ 0-7 row / 32-39 col, (f, x)]
                conv_t2 = apool.tile([16, 2 * NPOOL * 258], db)
                c3 = conv_t2[:].rearrange("p (b n w) -> p b n w", b=2, n=NPOOL)
                t75_t = apool.tile([16, 2 * NPOOL * 258], db)
                t753 = t75_t[:].rearrange("p (b n w) -> p b n w", b=2, n=NPOOL)
                rwF = apool.tile([16, 2 * NPOOL * 512], db)
                rwF5 = rwF[:].rearrange(
                    "p (b n x two) -> p b n x two", b=2, n=NPOOL, two=2)
                rwF_dram = dpool.tile([16, 2 * NPOOL * 512], db)
                r5d = rwF_dram[:].rearrange("f (b n x) -> f b n x", b=2, n=NPOOL)
                rfd = rwF_dram[:].rearrange("f (b n x) -> b n f x", b=2, n=NPOOL)
                rwF5v = rwF[:].rearrange("p (b n x) -> p b n x", b=2, n=NPOOL)
                psC_cm = tc.tile_pool(name="psConv", bufs=4, space="PSUM")
                psC_pool = psC_cm.__enter__()
                for ch in range(4):  # n-pair chunks: n in {2ch, 2ch+1}
                    n0 = 2 * ch
                    for b in range(2):
                        ps = psC_pool.tile([16, 2 * WP], dt, tag="conv")
                        # zero-init whole chunk (ktaps slot 12 = zeros)
                        nc.tensor.matmul(
                            ps[:], ktaps_t[:, 192:208], tpad3[:, n0:n0 + 2, 1:257],
                            start=True, stop=False, skip_group_check=True,
                        )
                        pieces = []
                        for i, (dn, dwp) in enumerate(taps_by_branch[b]):
                            nlo = max(n0, -dn)
                            nhi = min(n0 + 2, NPOOL - dn)
                            if nhi <= nlo:
                                continue
                            pieces.append((b * 6 + i, dn, dwp, nlo, nhi))
                        for k, (sl, dn, dwp, nlo, nhi) in enumerate(pieces):
                            nc.tensor.matmul(
                                ps[:, (nlo - n0) * WP:(nhi - n0) * WP],
                                ktaps_t[:, sl * 16:(sl + 1) * 16],
                                tpad3[:, nlo + dn:nhi + dn, 1 + dwp:257 + dwp],
                                start=False, stop=(k == len(pieces) - 1),
                                skip_group_check=True,
                            )
                        nc.scalar.activation(
                            out=c3[:, b, n0:n0 + 2, 1:257],
                            in_=ps[:],
                            func=mybir.ActivationFunctionType.Relu,
                            bias=bias_t[:, 0:1],
                        )
                    # edge replicate (W clamp), both branches of this chunk
                    nc.vector.tensor_copy(
                        c3[:, :, n0:n0 + 2, 0:1], c3[:, :, n0:n0 + 2, 1:2])
                    nc.vector.tensor_copy(
                        c3[:, :, n0:n0 + 2, 257:258], c3[:, :, n0:n0 + 2, 256:257])
                    # W-upsample this chunk on 16 f-lanes (per branch: the
                    # BIR tensor-scalar ops allow at most 3 canonical AP dims):
                    #   rw[., 2k]   = 0.25*pad[k]   + 0.75*pad[k+1]
                    #   rw[., 2k+1] = 0.25*pad[k+2] + 0.75*pad[k+1]
                    for b in range(2):
                        nc.vector.tensor_scalar_mul(
                            t753[:, b, n0:n0 + 2, :], c3[:, b, n0:n0 + 2, :], 0.75)
                        nc.vector.scalar_tensor_tensor(
                            out=rwF5[:, b, n0:n0 + 2, :, 0],
                            in0=c3[:, b, n0:n0 + 2, 0:256],
                            scalar=0.25,
                            in1=t753[:, b, n0:n0 + 2, 1:257],
                            op0=mybir.AluOpType.mult,
                            op1=mybir.AluOpType.add,
                        )
                        nc.vector.scalar_tensor_tensor(
                            out=rwF5[:, b, n0:n0 + 2, :, 1],
                            in0=c3[:, b, n0:n0 + 2, 2:258],
                            scalar=0.25,
                            in1=t753[:, b, n0:n0 + 2, 1:257],
                            op0=mybir.AluOpType.mult,
                            op1=mybir.AluOpType.add,
                        )
                    # bounce this chunk's rows: rwF -> DRAM -> rw partitions
                    nc.sync.dma_start(
                        out=r5d[:, :, n0:n0 + 2, :], in_=rwF5v[:, :, n0:n0 + 2, :])
                    for b in range(2):
                        pg = 32 * b
                        nc.sync.dma_start(
                            out=rw_t[pg + n0:pg + n0 + 2, :],
                            in_=rfd[b, n0:n0 + 2],
                        )
                psC_cm.__exit__(None, None, None)

            # ================= PASS B: H-upsample + combine + store =================
            # q-outer: one [128, 128x, 48ch] staging tile at a time (bufs=2 for
            # overlap), per-(q,b,fq) single-bank PSUM tiles (free dim 128)
            with (
                tc.tile_pool(name="passB", bufs=2) as bpool,
                tc.tile_pool(name="psB", bufs=2, space="PSUM") as psB,
            ):
                rwx = rw_t[:].rearrange("p (f x) -> p f x", x=W)
                for t in range(4):
                    fm3 = fmb_ts[t][:].rearrange("p (x c) -> p x c", c=C)
                    for q in range(4):
                        xs = 128 * q
                        outq_t = bpool.tile([128, 128 * CH_OUT], dt, tag="outq")
                        outq3 = outq_t[:].rearrange("p (x ch) -> p x ch", ch=CH_OUT)
                        nc.scalar.activation(
                            out=outq3[:, :, 0:16],
                            in_=fm3[:, xs:xs + 128, :],
                            func=mybir.ActivationFunctionType.Copy,
                        )
                        for b in range(2):
                            pg = 32 * b
                            lhsT = hup_t[pg:pg + 8, 128 * t:128 * (t + 1)]  # [8,128]
                            # one 4-bank PSUM tile [128, (16 f, 128 x)]; one
                            # matmul per bank (rhs free = 4f x 128x strided)
                            ps = psB.tile([128, 16 * 128], dt, tag="up")
                            psf = ps[:].rearrange("p (f x) -> p f x", x=128)
                            for fq in range(4):
                                nc.tensor.matmul(
                                    psf[:, 4 * fq:4 * (fq + 1), :],
                                    lhsT,
                                    rwx[pg:pg + 8, fq * 4:fq * 4 + 4, xs:xs + 128],
                                    start=True, stop=True,
                                )
                            psx = ps[:].rearrange("p (f x) -> p x f", x=128)
                            nc.vector.tensor_sub(
                                outq3[:, :, 16 * (b + 1):16 * (b + 2)],
                                fm3[:, xs:xs + 128, :],
                                psx[:],
                            )
                        nc.sync.dma_start(
                            out=out_d[128 * t:128 * (t + 1), xs:xs + 128, :],
                            in_=outq3,
                        )
    if compile:
        nc.compile()
    return nc


def _get_program():
    if "nc" not in _cache:
        _cache["nc"] = _build_program()
    return _cache["nc"]


def kernel(feature_map, kernel, bias):
    from concourse.bass_utils import run_bass_kernel_spmd

    feature_map = np.ascontiguousarray(feature_map, dtype=np.float32)
    kernel = np.ascontiguousarray(kernel, dtype=np.float32)
    bias = np.ascontiguousarray(bias, dtype=np.float32)
    B = feature_map.shape[0]
    assert B == 8

    poolw, hup, kt, bias2, _, _ = _host_consts(kernel, bias)
    nc = _get_program()
    in_maps = [
        {
            "feature_map": feature_map[b],
            "poolw": poolw,
            "hup": hup,
            "ktaps": kt,
            "bias2": bias2,
        }
        for b in range(B)
    ]
    res = run_bass_kernel_spmd(nc, in_maps, list(range(B)))
    out = np.stack([res.results[b]["out"] for b in range(B)])
    return out



# revision 19
# speedup vs baseline: 1.7633x; 1.1077x over previous
"""Trainium2 Bass kernel for nn_InpaintContextAttentionUnit.

Per-sample computation (B=8 samples -> 1 per NeuronCore):
  fm [512,512,16] -> avgpool(64x2) -> pooled [8,256,16]
  -> two masked 3x3 convs (middle row / middle col of kernel zeroed) + bias + relu
  -> bilinear upsample back to [512,512,16] (separable; half-pixel centers, edge clamp)
  -> out [512,512,48] = concat(fm, fm - row_up, fm - col_up)

Design (v2):
  - fm is read from HBM ONCE per core as a resident bf16 SBUF copy (SWDGE
    cast-DMA); pooling, the passthrough channels, and the subtract all read it
    (bf16 rounding of fm costs ~1e-2 abs vs the 0.109 tolerance)
  - pooling: PE matmul with a [128,2] block-mean matrix (H-reduce); W-pair add
    folded into a 2-matmul PSUM accumulation (even/odd x, strided rhs); all 4
    fm tiles accumulate into one [8, 4096] PSUM tile drained by one ACT copy
  - conv: per (branch, n-pair chunk): zero-init matmul + ~6 accumulating
    [16c,16f]x[16c,<=512] matmuls in PSUM; relu+bias on ACT; taps read from a
    wp-halo'd [16c, 8n x 258wp] buffer assembled via a DRAM bounce
  - W-upsample (x2, weights .25/.75): 2 strided scalar_tensor_tensor ops over an
    edge-replicated halo buffer
  - H-upsample (x64): PE matmul rw[8n, x] with host-built HUp interp matrix
    (row branch at partitions 0-7, col branch at 32-39 per base-partition rules)
  - combine: DVE subtract (fm - psum, strided APs) + ACT copy into interleaved
    [y, x, 48ch] staging tiles, contiguous 3 MiB DMAs out
  - the pooled->conv->upsample chain runs in bf16 (PE bf16 is ~4x faster than the
    fp32-emulation path); PSUM accumulation, fm passthrough, subtract, and the
    output stay fp32
All constant matrices are precomputed on host and passed as extra inputs.
"""

import numpy as np
import ml_dtypes

H, W, C, F = 512, 512, 16, 16
NPOOL = 8
WP = W // 2  # 256
CH_OUT = 3 * C  # 48

_cache = {}


def _host_consts(kernel, bias):
    """Build host-side constant matrices (bf16 for the PE-side constants)."""
    bf = ml_dtypes.bfloat16
    # pooling weights: [128, 32]; tile t uses columns 8t:8t+8, whose cols
    # 2t/2t+1 hold 1/128 (exact in bf16) on the matching 64-row block and all
    # other cols are zero -> a [128,8]-lhsT matmul per tile writes the full
    # 8-partition PSUM tile (base-partition rule) while accumulating only
    # its own pooled rows
    poolw = np.zeros((128, 32), np.float32)
    for t in range(4):
        poolw[:64, 8 * t + 2 * t] = 1.0 / 128.0
        poolw[64:, 8 * t + 2 * t + 1] = 1.0 / 128.0
    # H-upsample matrix: hup[n, y] = weight of pooled row n for output row y
    # (k/64 weights are exact in bf16)
    hup = np.zeros((NPOOL, H), np.float32)
    scale = H // NPOOL
    for y in range(H):
        yf = (y + 0.5) / scale - 0.5
        i0 = int(np.floor(yf))
        w = yf - i0
        hup[min(max(i0, 0), NPOOL - 1), y] += 1.0 - w
        hup[min(max(i0 + 1, 0), NPOOL - 1), y] += w
    hup2 = np.zeros((40, H), np.float32)
    hup2[0:8] = hup
    hup2[32:40] = hup  # col-branch copy at base partition 32
    # conv taps: branch 0 (row conv): K[dn+1, dwp+1]; branch 1 (col): K[dwp+1, dn+1]
    taps0 = [(dn, dwp) for dn in (-1, 1) for dwp in (-1, 0, 1)]
    taps1 = [(dn, dwp) for dwp in (-1, 1) for dn in (-1, 0, 1)]
    kt = np.zeros((16, 13 * 16), np.float32)  # [c, tap*16+f]; slot 12 = zeros
    for i, (dn, dwp) in enumerate(taps0):
        kt[:, i * 16:(i + 1) * 16] = kernel[dn + 1, dwp + 1]
    for i, (dn, dwp) in enumerate(taps1):
        kt[:, (6 + i) * 16:(7 + i) * 16] = kernel[dwp + 1, dn + 1]
    bias2 = np.ascontiguousarray(bias.reshape(16, 1)).astype(np.float32)
    return (poolw.astype(bf), hup2.astype(bf), kt.astype(bf), bias2, taps0, taps1)


def _build_program(compile=True):
    import concourse.bass as bass
    import concourse.bacc as bacc
    import concourse.mybir as mybir
    import concourse.tile as tile

    dt = mybir.dt.float32
    db = mybir.dt.bfloat16
    nc = bacc.Bacc()

    fm_d = nc.declare_dram_parameter("feature_map", [H, W, C], dt, isOutput=False)
    poolw_d = nc.declare_dram_parameter("poolw", [128, 32], db, isOutput=False)
    hup_d = nc.declare_dram_parameter("hup", [40, H], db, isOutput=False)
    ktaps_d = nc.declare_dram_parameter("ktaps", [16, 208], db, isOutput=False)
    bias_d = nc.declare_dram_parameter("bias2", [16, 1], dt, isOutput=False)
    out_d = nc.declare_dram_parameter("out", [H, W, CH_OUT], dt, isOutput=True)

    taps0 = [(dn, dwp) for dn in (-1, 1) for dwp in (-1, 0, 1)]
    taps1 = [(dn, dwp) for dwp in (-1, 1) for dn in (-1, 0, 1)]
    taps_by_branch = [taps0, taps1]

    with tile.TileContext(nc) as tc:
        with (
            tc.tile_pool(name="consts", bufs=1) as cpool,
            tc.tile_pool(name="persist", bufs=1) as ppool,
        ):
            # ---- load constants ----
            poolw_t = cpool.tile([128, 32], db)
            nc.sync.dma_start(out=poolw_t[:], in_=poolw_d[:])
            hup_t = cpool.tile([40, H], db)
            nc.sync.dma_start(out=hup_t[:], in_=hup_d[:])
            ktaps_t = cpool.tile([16, 208], db)
            nc.sync.dma_start(out=ktaps_t[:], in_=ktaps_d[:])
            bias_t = cpool.tile([16, 1], dt)
            nc.sync.dma_start(out=bias_t[:], in_=bias_d[:])

            # rw [40, (16 f, 512 x)] bf16: partitions 0-7 row-branch, 32-39 col-branch
            rw_t = ppool.tile([40, 16 * 512], db)

            # resident bf16 copy of fm (single HBM read serves pooling + pass B)
            fmb_ts = []
            for t in range(4):
                fmb_t = ppool.tile([128, W * C], db, tag=f"fmb{t}")
                fmb3 = fmb_t[:].rearrange("p (x c) -> p x c", c=C)
                nc.gpsimd.dma_start(out=fmb3, in_=fm_d[128 * t:128 * (t + 1)])
                fmb_ts.append(fmb_t)

            # ================= PASS A: pooling + conv + W-upsample =================
            with (
                tc.tile_pool(name="passA", bufs=1) as apool,
                tc.tile_pool(name="dram", bufs=1, space="DRAM") as dpool,
            ):
                # pooled_T [16 c, (8 n, 258 wp)] bf16, zero wp-halo; n-direction
                # zero-padding handled by clipped matmul n-ranges
                tpad_t = apool.tile([16, NPOOL * 258], db)
                tpad3 = tpad_t[:].rearrange("p (n w) -> p n w", w=258)

                with tc.tile_pool(name="psA", bufs=1, space="PSUM") as psA:
                    # all 8 pooled rows accumulate into one [8, 4096] PSUM tile;
                    # rhs free AP is (c, xp) so PSUM lands (j, c, xp)-major and
                    # the drain copy below runs with 32-elem contiguous runs
                    ps8 = psA.tile([8, 8 * 512], dt, tag="pool")
                    for t in range(4):
                        fmr = fmb_ts[t][:].rearrange(
                            "p (xp two c) -> p xp two c", two=2, c=16)
                        for j in range(8):  # 32-xp chunks -> N=512
                            for par in range(2):
                                nc.tensor.matmul(
                                    ps8[:, 512 * j:512 * (j + 1)],
                                    poolw_t[:, 8 * t:8 * (t + 1)],
                                    fmr[:, 32 * j:32 * (j + 1), par, :],
                                    start=(t == 0 and par == 0),
                                    stop=(t == 3 and par == 1),
                                    skip_group_check=True,
                                )
                    # drain PSUM (x, c)-major into stage (c, w)-major; per-j
                    # copies alternate DVE/ACT so the transpose cost halves
                    stage_t = apool.tile([NPOOL, 16 * WP], db)
                    stage4 = stage_t[:].rearrange(
                        "p (c j x) -> p c j x", c=16, j=8, x=32)
                    ps84 = ps8[:].rearrange(
                        "p (j x c) -> p c j x", j=8, x=32, c=16)
                    for j in range(8):
                        if j % 2 == 0:
                            nc.vector.tensor_copy(
                                stage4[:, :, j, :], ps84[:, :, j, :])
                        else:
                            nc.scalar.activation(
                                out=stage4[:, :, j, :], in_=ps84[:, :, j, :],
                                func=mybir.ActivationFunctionType.Copy)

                # pooled -> pooled_T (c to partitions) via DRAM bounce, adding
                # zero wp-halo columns (zeros sourced from hup rows 8-15, zero by
                # construction)
                ncw_dram = dpool.tile([NPOOL, 16 * 258], db)
                nd3 = ncw_dram[:].rearrange("n (c w) -> n c w", w=258)
                ncw3s = stage_t[:].rearrange("p (c w) -> p c w", w=WP)
                nc.sync.dma_start(out=nd3[:, :, 1:257], in_=ncw3s)
                zsrc = hup_d[8:16, 0:16]  # [8, 16] zeros
                nc.sync.dma_start(out=nd3[:, :, 0:1], in_=zsrc)
                nc.sync.dma_start(out=nd3[:, :, 257:258], in_=zsrc)
                ncwd3 = ncw_dram[:].rearrange("n (c w) -> c n w", w=258)
                nc.sync.dma_start(out=tpad3, in_=ncwd3)

                # ---- conv branches (chunk-major so rw rows stream out early) ----
                # conv output kept on 16 f-partitions with a wp-halo:
                # c3 [16 f, (b, n, 258 wp)]; W-upsample runs on the same 16-lane
                # layout BEFORE the n-to-partition transpose, then each chunk's
                # rows bounce via DRAM into rw [n @ 0-7 row / 32-39 col, (f, x)]
                conv_t2 = apool.tile([16, 2 * NPOOL * 258], db)
                c3 = conv_t2[:].rearrange("p (b n w) -> p b n w", b=2, n=NPOOL)
                t75_t = apool.tile([16, 2 * NPOOL * 258], db)
                t753 = t75_t[:].rearrange("p (b n w) -> p b n w", b=2, n=NPOOL)
                rwF = apool.tile([16, 2 * NPOOL * 512], db)
                rwF5 = rwF[:].rearrange(
                    "p (b n x two) -> p b n x two", b=2, n=NPOOL, two=2)
                rwF_dram = dpool.tile([16, 2 * NPOOL * 512], db)
                r5d = rwF_dram[:].rearrange("f (b n x) -> f b n x", b=2, n=NPOOL)
                rfd = rwF_dram[:].rearrange("f (b n x) -> b n f x", b=2, n=NPOOL)
                rwF5v = rwF[:].rearrange("p (b n x) -> p b n x", b=2, n=NPOOL)
                psC_cm = tc.tile_pool(name="psConv", bufs=4, space="PSUM")
                psC_pool = psC_cm.__enter__()
                for ch in range(4):  # n-pair chunks: n in {2ch, 2ch+1}
                    n0 = 2 * ch
                    for b in range(2):
                        ps = psC_pool.tile([16, 2 * WP], dt, tag="conv")
                        # zero-init whole chunk (ktaps slot 12 = zeros)
                        nc.tensor.matmul(
                            ps[:], ktaps_t[:, 192:208], tpad3[:, n0:n0 + 2, 1:257],
                            start=True, stop=False, skip_group_check=True,
                        )
                        pieces = []
                        for i, (dn, dwp) in enumerate(taps_by_branch[b]):
                            nlo = max(n0, -dn)
                            nhi = min(n0 + 2, NPOOL - dn)
                            if nhi <= nlo:
                                continue
                            pieces.append((b * 6 + i, dn, dwp, nlo, nhi))
                        for k, (sl, dn, dwp, nlo, nhi) in enumerate(pieces):
                            nc.tensor.matmul(
                                ps[:, (nlo - n0) * WP:(nhi - n0) * WP],
                                ktaps_t[:, sl * 16:(sl + 1) * 16],
                                tpad3[:, nlo + dn:nhi + dn, 1 + dwp:257 + dwp],
                                start=False, stop=(k == len(pieces) - 1),
                                skip_group_check=True,
                            )
                        nc.scalar.activation(
                            out=c3[:, b, n0:n0 + 2, 1:257],
                            in_=ps[:],
                            func=mybir.ActivationFunctionType.Relu,
                            bias=bias_t[:, 0:1],
                        )
                    # edge replicate (W clamp), both branches of this chunk
                    nc.vector.tensor_copy(
                        c3[:, :, n0:n0 + 2, 0:1], c3[:, :, n0:n0 + 2, 1:2])
                    nc.vector.tensor_copy(
                        c3[:, :, n0:n0 + 2, 257:258], c3[:, :, n0:n0 + 2, 256:257])
                    # W-upsample this chunk on 16 f-lanes (per branch: the
                    # BIR tensor-scalar ops allow at most 3 canonical AP dims):
                    #   rw[., 2k]   = 0.25*pad[k]   + 0.75*pad[k+1]
                    #   rw[., 2k+1] = 0.25*pad[k+2] + 0.75*pad[k+1]
                    for b in range(2):
                        nc.vector.tensor_scalar_mul(
                            t753[:, b, n0:n0 + 2, :], c3[:, b, n0:n0 + 2, :], 0.75)
                        nc.vector.scalar_tensor_tensor(
                            out=rwF5[:, b, n0:n0 + 2, :, 0],
                            in0=c3[:, b, n0:n0 + 2, 0:256],
                            scalar=0.25,
                            in1=t753[:, b, n0:n0 + 2, 1:257],
                            op0=mybir.AluOpType.mult,
                            op1=mybir.AluOpType.add,
                        )
                        nc.vector.scalar_tensor_tensor(
                            out=rwF5[:, b, n0:n0 + 2, :, 1],
                            in0=c3[:, b, n0:n0 + 2, 2:258],
                            scalar=0.25,
                            in1=t753[:, b, n0:n0 + 2, 1:257],
                            op0=mybir.AluOpType.mult,
                            op1=mybir.AluOpType.add,
                        )
                    # bounce this chunk's rows: rwF -> DRAM -> rw partitions
                    nc.sync.dma_start(
                        out=r5d[:, :, n0:n0 + 2, :], in_=rwF5v[:, :, n0:n0 + 2, :])
                    for b in range(2):
                        pg = 32 * b
                        nc.sync.dma_start(
                            out=rw_t[pg + n0:pg + n0 + 2, :],
                            in_=rfd[b, n0:n0 + 2],
                        )
                psC_cm.__exit__(None, None, None)

            # ================= PASS B: H-upsample + combine + store =================
            # q-outer: one [128, 128x, 48ch] staging tile at a time (bufs=2 for
            # overlap), per-(q,b,fq) single-bank PSUM tiles (free dim 128)
            with (
                tc.tile_pool(name="passB", bufs=2) as bpool,
                tc.tile_pool(name="psB", bufs=2, space="PSUM") as psB,
            ):
                rwx = rw_t[:].rearrange("p (f x) -> p f x", x=W)
                for t in range(4):
                    fm3 = fmb_ts[t][:].rearrange("p (x c) -> p x c", c=C)
                    for q in range(4):
                        xs = 128 * q
                        outq_t = bpool.tile([128, 128 * CH_OUT], dt, tag="outq")
                        outq3 = outq_t[:].rearrange("p (x ch) -> p x ch", ch=CH_OUT)
                        nc.scalar.activation(
                            out=outq3[:, :, 0:16],
                            in_=fm3[:, xs:xs + 128, :],
                            func=mybir.ActivationFunctionType.Copy,
                        )
                        for b in range(2):
                            pg = 32 * b
                            # t=0 only blends pooled rows n<=2 (hup cols 0:128
                            # are zero for n>=3), so contract over n 0-3 only --
                            # tile 0's output then depends just on conv chunks
                            # 0-1 and its writes start while chunks 2-3 run
                            nk = 4 if t == 0 else 8
                            lhsT = hup_t[pg:pg + nk, 128 * t:128 * (t + 1)]
                            # one 4-bank PSUM tile [128, (16 f, 128 x)]; one
                            # matmul per bank (rhs free = 4f x 128x strided)
                            ps = psB.tile([128, 16 * 128], dt, tag="up")
                            psf = ps[:].rearrange("p (f x) -> p f x", x=128)
                            for fq in range(4):
                                nc.tensor.matmul(
                                    psf[:, 4 * fq:4 * (fq + 1), :],
                                    lhsT,
                                    rwx[pg:pg + nk, fq * 4:fq * 4 + 4, xs:xs + 128],
                                    start=True, stop=True,
                                )
                            psx = ps[:].rearrange("p (f x) -> p x f", x=128)
                            nc.vector.tensor_sub(
                                outq3[:, :, 16 * (b + 1):16 * (b + 2)],
                                fm3[:, xs:xs + 128, :],
                                psx[:],
                            )
                        nc.sync.dma_start(
                            out=out_d[128 * t:128 * (t + 1), xs:xs + 128, :],
                            in_=outq3,
                        )
    if compile:
        nc.compile()
    return nc


def _get_program():
    if "nc" not in _cache:
        _cache["nc"] = _build_program()
    return _cache["nc"]


def kernel(feature_map, kernel, bias):
    from concourse.bass_utils import run_bass_kernel_spmd

    feature_map = np.ascontiguousarray(feature_map, dtype=np.float32)
    kernel = np.ascontiguousarray(kernel, dtype=np.float32)
    bias = np.ascontiguousarray(bias, dtype=np.float32)
    B = feature_map.shape[0]
    assert B == 8

    poolw, hup, kt, bias2, _, _ = _host_consts(kernel, bias)
    nc = _get_program()
    in_maps = [
        {
            "feature_map": feature_map[b],
            "poolw": poolw,
            "hup": hup,
            "ktaps": kt,
            "bias2": bias2,
        }
        for b in range(B)
    ]
    res = run_bass_kernel_spmd(nc, in_maps, list(range(B)))
    out = np.stack([res.results[b]["out"] for b in range(B)])
    return out

